# revision 100
# baseline (speedup 1.0000x reference)
"""LIPAR segment attention kernel for TRN2, 8 NeuronCores.

Problem (hardcoded): B=4, N=4096, DIM=768, H=12 heads, DH=64, S=16 segments
of M=256 tokens. q = x@Wq (scaled, rotary), kv = x@Wkv (rotary, shared K==V).
Segment t>=1 attends to segments [t-1, t]; segment 0 attends to itself.
Output projection Wo/bo for segments >=1, Wo0/bo0 for segment 0.

Sharding: the (b h) = 48 fused axis is split 8 ways -> 6 heads per core,
one batch per core pair. Each core computes a partial output projection from
its 384 feature rows; the host sums the two partial (768, 4096) results per
batch and adds biases.

v2 design (vs v1): all matmul inputs bf16 (1 PE cycle/row instead of fp32's
4); AV computed in natural layout (queries on partitions) so softmax
normalization is a per-partition scale instead of a PE broadcast matmul;
rotate-half done with SBUF->SBUF partition-shift DMAs (sign baked into the
sinT table) instead of a PE permutation matmul; kvn/outT layout transposes
on PE in bf16 with batched PSUM->SBUF copies; exp batched into (128,1024)
tiles; and the whole schedule software-pipelined per head: iteration b
interleaves QK(b), AV(b-1), kvn(b) transposes and the b+1 projections so
exp (ACT-only, the slowest chain) always has PE work overlapping it, with
output stores deferred one iteration so they enter the 16-deep HWDGE DMA
ring already-ready (a dep-blocked DMA there convoys everything behind it).
"""

import numpy as np

B, N, DIM = 4, 4096, 768
H = 12
DH = 64
S = 16
M = 256
SCALE = DH**-0.5

HPC = 6            # heads per core
FPC = HPC * DH     # 384 features per core
KC = DIM // 128    # 6 contraction chunks
NB = 8             # token blocks
TB = N // NB       # 512 tokens per block (2 segments)
NCORES = 8


def _host_tables():
    """cosT/sinT (128, N) in the 2-head-stacked transposed layout, with the
    rotate-half sign baked into sinT."""
    inv_freq = 1.0 / (10000.0 ** (np.arange(0, DH, 2, dtype=np.float64) / DH))
    t = np.arange(N, dtype=np.float64)
    freqs = np.outer(inv_freq, t)            # (32, N)
    r = np.arange(128)
    fidx = r % 32
    cosT = np.cos(freqs)[fidx].astype(np.float32)            # (128, N)
    sign = np.where((r % 64) < 32, -1.0, 1.0)[:, None]
    sinT = (sign * np.sin(freqs)[fidx]).astype(np.float32)   # (128, N)
    return cosT, sinT


def _build_nc():
    import concourse.bass as bass
    import concourse.bacc as bacc
    import concourse.tile as tile
    from concourse import mybir
    from concourse.masks import make_identity
    from contextlib import ExitStack

    f32 = mybir.dt.float32
    bf16 = mybir.dt.bfloat16
    EXP = mybir.ActivationFunctionType.Exp

    nc = bacc.Bacc("TRN2", target_bir_lowering=False)
    dbg = {}
    if DEBUG:
        dbg["qrot"] = nc.dram_tensor("d_qrot", [128, 3, TB], bf16, kind="ExternalOutput")
        dbg["kvrot"] = nc.dram_tensor("d_kvrot", [128, 3, TB], bf16, kind="ExternalOutput")
        dbg["kvn"] = nc.dram_tensor("d_kvn", [128, HPC, DH + 1], bf16, kind="ExternalOutput")
        dbg["p1"] = nc.dram_tensor("d_p1", [128, 1024], bf16, kind="ExternalOutput")
        dbg["p0b1"] = nc.dram_tensor("d_p0b1", [128, 1024], bf16, kind="ExternalOutput")
        dbg["on"] = nc.dram_tensor("d_on", [128, HPC, DH], bf16, kind="ExternalOutput")
        dbg["outT"] = nc.dram_tensor("d_outT", [128, 3, TB], bf16, kind="ExternalOutput")
    xT = nc.dram_tensor("xT", [DIM, N], bf16, kind="ExternalInput")
    wq = nc.dram_tensor("wq", [DIM, FPC], bf16, kind="ExternalInput")
    wkv = nc.dram_tensor("wkv", [DIM, FPC], bf16, kind="ExternalInput")
    wo = nc.dram_tensor("wo", [FPC, DIM], bf16, kind="ExternalInput")
    wo0 = nc.dram_tensor("wo0", [FPC, DIM], bf16, kind="ExternalInput")
    cosT = nc.dram_tensor("cosT", [128, N], bf16, kind="ExternalInput")
    sinT = nc.dram_tensor("sinT", [128, N], bf16, kind="ExternalInput")
    outpT = nc.dram_tensor("outpT", [DIM, N], bf16, kind="ExternalOutput")

    outpT_r = outpT.rearrange("(c p) n -> p c n", p=128)
    xT_r = xT.rearrange("(c p) n -> p c n", p=128)

    with tile.TileContext(nc) as tc, ExitStack() as ctx:
        consts = ctx.enter_context(tc.tile_pool(name="consts", bufs=1))
        xpool = ctx.enter_context(tc.tile_pool(name="xpool", bufs=3))
        cspool = ctx.enter_context(tc.tile_pool(name="cspool", bufs=3))
        rawpool = ctx.enter_context(tc.tile_pool(name="rawpool", bufs=8))
        shfpool = ctx.enter_context(tc.tile_pool(name="shfpool", bufs=8))
        tmppool = ctx.enter_context(tc.tile_pool(name="tmppool", bufs=8))
        qrpool = ctx.enter_context(tc.tile_pool(name="qrpool", bufs=3))
        kvrpool = ctx.enter_context(tc.tile_pool(name="kvrpool", bufs=4))
        kvnpool = ctx.enter_context(tc.tile_pool(name="kvnpool", bufs=16))
        ppool = ctx.enter_context(tc.tile_pool(name="ppool", bufs=26))
        rcpool = ctx.enter_context(tc.tile_pool(name="rcpool", bufs=6))
        onpool = ctx.enter_context(tc.tile_pool(name="onpool", bufs=10))
        otpool = ctx.enter_context(tc.tile_pool(name="otpool", bufs=4))
        prpool = ctx.enter_context(tc.tile_pool(name="prpool", bufs=4))

        mmps = ctx.enter_context(tc.tile_pool(name="mmps", bufs=3, space="PSUM"))
        stps = ctx.enter_context(tc.tile_pool(name="stps", bufs=2, space="PSUM"))
        avps = ctx.enter_context(tc.tile_pool(name="avps", bufs=1, space="PSUM"))

        # resident constants -- emitted in first-use order so block 0's x/q
        # projection isn't stuck behind the output-projection weight loads
        wq_sb = consts.tile([128, KC, FPC], bf16, tag="wq_sb")
        wkv_sb = consts.tile([128, KC, FPC], bf16, tag="wkv_sb")
        wo_sb = consts.tile([128, 3, DIM], bf16, tag="wo_sb")
        wo0_sb = consts.tile([128, 3, DIM], bf16, tag="wo0_sb")
        ident = consts.tile([128, 128], bf16, tag="ident")
        make_identity(nc, ident)

        def load_consts_head():
            nc.sync.dma_start(
                out=wq_sb, in_=wq.rearrange("(c p) m -> p c m", p=128)
            )

        def load_consts_tail():
            nc.sync.dma_start(
                out=wo_sb, in_=wo.rearrange("(c p) m -> p c m", p=128)
            )
            nc.sync.dma_start(
                out=wo0_sb, in_=wo0.rearrange("(c p) m -> p c m", p=128)
            )

        class ProjState:
            """Projection + rotary for one block, emitted in 6 interleavable
            parts: (q,t0..2), (kv,t0..2). ps_part projects and starts the
            cos path; shps_part does the rotate-half matmul and finishes."""

            def __init__(self, b, fast=False, split_x=False):
                self.fast = fast
                n0 = b * TB
                self.xt = xpool.tile([128, KC, TB], bf16, tag="xt")
                if split_x:
                    # halve the first x load so block 0's projection can
                    # start as soon as the first contraction chunks land
                    nc.sync.dma_start(
                        out=self.xt[:, 0:3, :], in_=xT_r[:, 0:3, n0 : n0 + TB]
                    )
                    nc.sync.dma_start(
                        out=self.xt[:, 3:KC, :], in_=xT_r[:, 3:KC, n0 : n0 + TB]
                    )
                else:
                    nc.sync.dma_start(out=self.xt, in_=xT_r[:, :, n0 : n0 + TB])
                self.cosb = cspool.tile([128, TB], bf16, tag="cosb")
                nc.sync.dma_start(out=self.cosb, in_=cosT[:, n0 : n0 + TB])
                self.sinb = cspool.tile([128, TB], bf16, tag="sinb")
                nc.sync.dma_start(out=self.sinb, in_=sinT[:, n0 : n0 + TB])
                self.qrot = qrpool.tile([128, 3, TB], bf16, tag="qrot")
                self.kvrot = kvrpool.tile([128, 3, TB], bf16, tag="kvrot")
                # kv parts first: the kvn transposes at the next iteration's
                # start need all kvrot rows, while qrot's later rows aren't
                # read until QK reaches heads 4-5 mid-iteration.
                self.parts = [
                    (wkv_sb, self.kvrot, t) for t in range(3)
                ] + [(wq_sb, self.qrot, t) for t in range(3)]
                self.state = {}

            def ps_part(self, i):
                wsb, rot, t = self.parts[i]
                ps = mmps.tile([128, TB], f32, tag="mm")
                for c in range(KC):
                    nc.tensor.matmul(
                        ps,
                        lhsT=wsb[:, c, t * 128 : (t + 1) * 128],
                        rhs=self.xt[:, c, :],
                        start=(c == 0),
                        stop=(c == KC - 1),
                    )
                raw = rawpool.tile([128, TB], bf16, tag="raw")
                nc.vector.tensor_copy(raw, ps)
                # rotate-half via partition-shift DMAs (sign baked into sinT)
                shf = shfpool.tile([128, TB], bf16, tag="shf")
                for a in range(4):
                    d0 = a * 32
                    s0 = (a ^ 1) * 32
                    nc.sync.dma_start(
                        out=shf[d0 : d0 + 32, :], in_=raw[s0 : s0 + 32, :]
                    )
                tmp1 = tmppool.tile([128, TB], bf16, tag="tmp1")
                eng = nc.vector if self.fast else nc.gpsimd
                eng.tensor_mul(tmp1, raw, self.cosb)
                self.state[i] = (shf, tmp1)

            def shps_part(self, i):
                wsb, rot, t = self.parts[i]
                shf, tmp1 = self.state.pop(i)
                tmp2 = tmppool.tile([128, TB], bf16, tag="tmp2")
                nc.vector.tensor_mul(tmp2, shf, self.sinb)
                eng = nc.vector if self.fast else nc.gpsimd
                eng.tensor_add(rot[:, t, :], tmp1, tmp2)

        def emit_qk_head(b, h, qrot, kvrot, kvrot_prev):
            """QK^T (keys on partitions) + exp for head h of block b, batched
            into two (128,1024) score tiles.
            p0 = [P2xq0|P3xq0|C2xq1|C3xq1] (prev-seg and seg-2b+1 keys),
            p1 = [C0x(q0q1)|C1x(q0q1)] (seg-2b keys).
            b=0: p0 = [C2xq1|C3xq1] only (128,512)."""
            t, m_ = divmod(h, 2)
            r0 = m_ * DH
            kvl = kvrot[r0 : r0 + DH, t, :]
            ql = qrot[r0 : r0 + DH, t, :]
            if b > 0:
                kvpl = kvrot_prev[r0 : r0 + DH, t, :]
                st0 = stps.tile([128, 1024], f32, tag="st")
                nc.tensor.matmul(
                    st0[:, 0:256], lhsT=kvpl[:, 256:384],
                    rhs=ql[:, 0:256], start=True, stop=True,
                )
                nc.tensor.matmul(
                    st0[:, 256:512], lhsT=kvpl[:, 384:512],
                    rhs=ql[:, 0:256], start=True, stop=True,
                )
                nc.tensor.matmul(
                    st0[:, 512:768], lhsT=kvl[:, 256:384],
                    rhs=ql[:, 256:512], start=True, stop=True,
                )
                nc.tensor.matmul(
                    st0[:, 768:1024], lhsT=kvl[:, 384:512],
                    rhs=ql[:, 256:512], start=True, stop=True,
                )
                p0 = ppool.tile([128, 1024], bf16, tag="p")
                nc.scalar.activation(p0, st0, EXP)
            else:
                st0 = stps.tile([128, 1024], f32, tag="st")
                nc.tensor.matmul(
                    st0[:, 0:256], lhsT=kvl[:, 256:384],
                    rhs=ql[:, 256:512], start=True, stop=True,
                )
                nc.tensor.matmul(
                    st0[:, 256:512], lhsT=kvl[:, 384:512],
                    rhs=ql[:, 256:512], start=True, stop=True,
                )
                p0 = ppool.tile([128, 1024], bf16, tag="p")
                nc.scalar.activation(p0[:, 0:512], st0[:, 0:512], EXP)
            st1 = stps.tile([128, 1024], f32, tag="st")
            nc.tensor.matmul(
                st1[:, 0:512], lhsT=kvl[:, 0:128],
                rhs=ql[:, 0:512], start=True, stop=True,
            )
            nc.tensor.matmul(
                st1[:, 512:1024], lhsT=kvl[:, 128:256],
                rhs=ql[:, 0:512], start=True, stop=True,
            )
            p1 = ppool.tile([128, 1024], bf16, tag="p")
            nc.scalar.activation(p1, st1, EXP)
            return (p0, p1)

        def emit_av_qc(b, qc, pts, kvn, kvn_prev, outT, alt_pool=False):
            """AV for query chunk qc of block b in natural layout (queries on
            partitions, ones column -> denominator), then normalize and
            DMA-transpose into outT."""
            if alt_pool:
                av = mmps.tile([128, HPC, DH + 1], f32, tag="mm")
            else:
                av = avps.tile([128, HPC, DH + 1], f32, tag="av")
            for h in range(HPC):
                p0, p1 = pts[h]
                if b > 0:
                    if qc < 2:
                        chunks = [
                            (p0, qc * 128, kvn_prev[2]),
                            (p0, 256 + qc * 128, kvn_prev[3]),
                            (p1, qc * 128, kvn[0]),
                            (p1, 512 + qc * 128, kvn[1]),
                        ]
                    else:
                        qq = (qc - 2) * 128
                        chunks = [
                            (p1, 256 + qq, kvn[0]),
                            (p1, 768 + qq, kvn[1]),
                            (p0, 512 + qq, kvn[2]),
                            (p0, 768 + qq, kvn[3]),
                        ]
                else:
                    if qc < 2:
                        chunks = [
                            (p1, qc * 128, kvn[0]),
                            (p1, 512 + qc * 128, kvn[1]),
                        ]
                    else:
                        qq = (qc - 2) * 128
                        chunks = [
                            (p1, 256 + qq, kvn[0]),
                            (p1, 768 + qq, kvn[1]),
                            (p0, qq, kvn[2]),
                            (p0, 256 + qq, kvn[3]),
                        ]
                for ci, (p_, poff, kvn_t) in enumerate(chunks):
                    nc.tensor.matmul(
                        av[:, h, :],
                        lhsT=p_[:, poff : poff + 128],
                        rhs=kvn_t[:, h, :],
                        start=(ci == 0),
                        stop=(ci == len(chunks) - 1),
                    )
            rcp = rcpool.tile([128, HPC, 1], f32, tag="rcp")
            nc.vector.reciprocal(rcp, av[:, :, DH : DH + 1])
            on = onpool.tile([128, HPC, DH], bf16, tag="on")
            nc.vector.tensor_mul(
                on, av[:, :, 0:DH], rcp.broadcast_to([128, HPC, DH])
            )
            if DEBUG and b == 0 and qc == 0:
                nc.sync.dma_start(out=dbg["on"][:, :, :], in_=on)
            return on

        def emit_outT(on, qc, outT):
            """Transpose a normalized AV chunk into outT. Deferred from
            emit_av_qc so PE doesn't wait on the DVE recip/norm chain."""
            tp = mmps.tile([128, 3, 128], bf16, tag="mm")
            for fc in range(3):
                nc.tensor.transpose(
                    tp[:, fc, :], on[:, 2 * fc : 2 * fc + 2, :], ident
                )
            dst = outT[:, :, qc * 128 : (qc + 1) * 128]
            if qc % 2 == 0:
                nc.scalar.copy(dst, tp)
            else:
                nc.vector.tensor_copy(dst, tp)

        def emit_outproj(b, outT):
            """Output projection (partial: this core's 384 features).
            Returns the prj tiles; the DMA-out is deferred to the next
            iteration so it enters the DMA ring already-ready (a dep-blocked
            DMA in the 16-deep HWDGE ring convoys everything behind it)."""
            n0 = b * TB
            if b == 0:
                ranges = [(0, M, wo0_sb), (M, TB, wo_sb)]
            else:
                ranges = [(0, TB, wo_sb)]
            prj = prpool.tile([128, 6, TB], bf16, tag="prj")
            for oc in range(6):
                pps = mmps.tile([128, TB], f32, tag="mm")
                for (a0, a1, wsb) in ranges:
                    for t in range(3):
                        nc.tensor.matmul(
                            pps[:, a0:a1],
                            lhsT=wsb[:, t, oc * 128 : (oc + 1) * 128],
                            rhs=outT[:, t, a0:a1],
                            start=(t == 0),
                            stop=(t == 2),
                        )
                if oc % 2 == 0:
                    nc.scalar.copy(prj[:, oc, :], pps)
                else:
                    nc.vector.tensor_copy(prj[:, oc, :], pps)
            return prj

        def emit_stores(b, prj):
            n0 = b * TB
            nc.sync.dma_start(out=outpT_r[:, :, n0 : n0 + TB], in_=prj)

        # Software pipeline, one iteration ahead on projections and one
        # behind on AV/output.  Iteration b interleaves, per head h:
        # QK(h, b) [+exp], AV(qc=h, b-1), projection part h of block b+1 --
        # so exp (ACT-only, the slowest chain) always has PE work overlapping
        # it and no engine queue convoys.
        nc.sync.dma_start(out=wkv_sb, in_=wkv.rearrange("(c p) m -> p c m", p=128))
        proj = ProjState(0, fast=True)
        load_consts_head()
        for i in range(6):
            proj.ps_part(i)
            proj.shps_part(i)
        qrot, kvrot = proj.qrot, proj.kvrot
        kvrot_prev = None
        kvn_hist = {}   # block -> kvn tiles
        pts_prev = None
        outT_prev = None

        prjs_pend = None  # (block, prj tiles) awaiting DMA-out

        for b in range(NB):
            if b == 0:
                load_consts_tail()
            # stores for block b-2's output projection (already-ready DMAs)
            if prjs_pend is not None:
                emit_stores(*prjs_pend)
                prjs_pend = None
            # kv natural layout (tokens on partitions) via PE transposes,
            # spread across the h loop (emit_kvn_part below)
            kvn = []
            for cc in range(4):
                kt = kvnpool.tile([128, HPC, DH + 1], bf16, tag="kvn")
                nc.gpsimd.memset(kt[:, :, DH : DH + 1], 1.0)
                kvn.append(kt)
            kvn_hist[b] = kvn

            def emit_kvn_chunk(cc):
                tps = mmps.tile([128, 3, 128], bf16, tag="mm")
                for t in range(3):
                    nc.tensor.transpose(
                        tps[:, t, :], kvrot[:, t, cc * 128 : (cc + 1) * 128],
                        ident,
                    )
                dst = kvn[cc][:, :, 0:DH]
                src = tps.rearrange("p t (a b) -> p (t a) b", a=2)
                nc.scalar.copy(dst, src)
            if DEBUG and b == 0:
                nc.sync.dma_start(out=dbg["qrot"][:, :, :], in_=qrot)
                nc.sync.dma_start(out=dbg["kvrot"][:, :, :], in_=kvrot)

            if b > 0:
                outT_prev = otpool.tile([128, 3, TB], bf16, tag="outT")
            proj = ProjState(b + 1, fast=(b <= 1)) if b + 1 < NB else None
            pts = []
            on_pend = []
            for h in range(HPC):
                pts.append(emit_qk_head(b, h, qrot, kvrot, kvrot_prev))
                if DEBUG and h == 0:
                    if b == 0:
                        nc.sync.dma_start(out=dbg["p1"][:, :], in_=pts[0][1])
                    elif b == 1:
                        nc.sync.dma_start(out=dbg["p0b1"][:, :], in_=pts[0][0])
                if b > 0 and on_pend:
                    emit_outT(*on_pend.pop(0), outT_prev)
                if b > 0 and h < 4:
                    on_pend.append((emit_av_qc(
                        b - 1, h, pts_prev, kvn_hist[b - 1],
                        kvn_hist.get(b - 2), outT_prev), h))
                if h < 4:
                    emit_kvn_chunk(h)
                if proj is not None:
                    proj.ps_part(h)
                    if h >= 1:
                        proj.shps_part(h - 1)
            if proj is not None:
                proj.shps_part(5)
            while on_pend:
                emit_outT(*on_pend.pop(0), outT_prev)
            if DEBUG and b == 0:
                nc.sync.dma_start(out=dbg["kvn"][:, :, :], in_=kvn[0])

            # output projection for previous block
            if b > 0:
                if DEBUG and b == 1:
                    nc.sync.dma_start(out=dbg["outT"][:, :, :], in_=outT_prev)
                prjs_pend = (b - 1, emit_outproj(b - 1, outT_prev))
                kvn_hist.pop(b - 2, None)

            kvrot_prev = kvrot
            pts_prev = pts
            if proj is not None:
                qrot, kvrot = proj.qrot, proj.kvrot

        # epilogue: AV + output projection for the last block
        if prjs_pend is not None:
            emit_stores(*prjs_pend)
        outT_last = otpool.tile([128, 3, TB], bf16, tag="outT")
        ep_pend = []
        for qc in range(4):
            if len(ep_pend) > 1:
                emit_outT(*ep_pend.pop(0), outT_last)
            ep_pend.append((emit_av_qc(
                NB - 1, qc, pts_prev, kvn_hist[NB - 1],
                kvn_hist.get(NB - 2), outT_last, alt_pool=(qc % 2 == 1)), qc))
        while ep_pend:
            emit_outT(*ep_pend.pop(0), outT_last)
        prj_last = emit_outproj(NB - 1, outT_last)
        n0_last = (NB - 1) * TB
        for oc in range(6):
            nc.sync.dma_start(
                out=outpT_r[:, oc, n0_last : n0_last + TB],
                in_=prj_last[:, oc, :],
            )

    nc.compile()
    return nc


_CACHE = {}
TRACE = False
DEBUG = False


def kernel(x, Wq, Wkv, Wo, bo, Wo0, bo0):
    import ml_dtypes
    from concourse.bass_utils import run_bass_kernel_spmd

    bfdt = np.dtype(ml_dtypes.bfloat16)

    x = np.asarray(x, dtype=np.float32)
    Wq = np.asarray(Wq, dtype=np.float32)
    Wkv = np.asarray(Wkv, dtype=np.float32)
    Wo = np.asarray(Wo, dtype=np.float32)
    bo = np.asarray(bo, dtype=np.float32)
    Wo0 = np.asarray(Wo0, dtype=np.float32)
    bo0 = np.asarray(bo0, dtype=np.float32)

    cosT, sinT = _host_tables()
    Wq_s = (Wq * SCALE).astype(np.float32)

    xTs = [np.ascontiguousarray(x[b_].T).astype(bfdt) for b_ in range(B)]
    in_maps = []
    for ci in range(NCORES):
        b_, hi = ci // 2, ci % 2
        fsl = slice(hi * FPC, (hi + 1) * FPC)
        in_maps.append(
            {
                "xT": xTs[b_],
                "wq": np.ascontiguousarray(Wq_s[:, fsl]).astype(bfdt),
                "wkv": np.ascontiguousarray(Wkv[:, fsl]).astype(bfdt),
                "wo": np.ascontiguousarray(Wo[fsl, :]).astype(bfdt),
                "wo0": np.ascontiguousarray(Wo0[fsl, :]).astype(bfdt),
                "cosT": cosT.astype(bfdt),
                "sinT": sinT.astype(bfdt),
            }
        )

    if "nc" not in _CACHE:
        _CACHE["nc"] = _build_nc()
    nc = _CACHE["nc"]

    res = run_bass_kernel_spmd(
        nc, in_maps, core_ids=list(range(NCORES)), trace=TRACE
    )
    _CACHE["last"] = res
    parts = [np.asarray(r["outpT"], dtype=np.float32) for r in res.results]

    out = np.empty((B, N, DIM), dtype=np.float32)
    bias = np.empty((N, DIM), dtype=np.float32)
    bias[:M] = bo0
    bias[M:] = bo
    for b_ in range(B):
        acc = parts[2 * b_] + parts[2 * b_ + 1]      # (768, 4096)
        out[b_] = acc.T + bias
    return out
